# revision 18
# baseline (speedup 1.0000x reference)
"""Trainium2 kernel for nn_COSSIMMLP (gnn_message_passing).

reference semantics:
    src = prop_state[b, mask[...,0]]; dst = prop_state[b, mask[...,1]]
    vals = sigmoid(cossim(src, dst))          # [B, E]
    adj[b, i, j] = vals; adj[b, j, i] = vals  # dense [B, N, N]

Every scatter write at position (r, c) carries the identical value
sigmoid(cos(s_r, s_c)), so the output is exactly

    adj = sigmoid(S_hat @ S_hat.T + Madd),  Madd = 0 at edge positions,
                                                   -240 elsewhere

with S_hat the eps-clamp-normalized rows.  sigmoid(x - 240) underflows to 0 in
f32, so non-edges are (numerically exact) zero.

Implementation highlights (167 us first-working -> this version):
  * gram matmul in fp8 DoubleRow perf mode (K=256 in one pass)
  * additive mask shipped as 1 bit/entry, expanded on the vector engine:
    one u16 tensor_scalar (shift + AND against 0x4040) per 512-column
    bit-plane yields bytes {0x00, 0x40} = fp8 {0, 2.0}; an identity scaled
    by -120 folds them into PSUM (masked entries get -240 before sigmoid).
  * f16 output tile + f16 HBM store (host widens to f32)
  * f16 prop input (host narrows; norm math still f32 on device)
  * inverse norms via a DVE-only Quake rsqrt (u32 exponent trick + 2 Newton
    steps) so the scalar engine never loads the sqrt table: its sigmoid
    table is warmed once at t0 and the 8.4M-element sigmoid train - which
    paces the whole phase B - runs back-to-back with no table switches
  * all 16 left-half (columns 0..2047) row tiles are computed first, then
    the right halves: the second batch of transposes and all remaining
    mask expansions hide completely under the sigmoid train

Sharding: 8 cores = 4 batches x 2 row-halves.  Each core computes a
[2048, 4096] slab of one batch's adjacency.  Per-core node order is rolled
by the row offset so that a single SPMD program serves all cores; the host
un-rolls output columns.
"""

import numpy as np
import ml_dtypes

B, N, D, E = 4, 4096, 256, 131072
NH = N // 2          # rows per core
P = 128              # partitions
NT = N // P          # 32 node tiles
MT = NH // P         # 16 row tiles per core
GRP = 8              # node tiles per phase-A group
EPS = 1e-8

_prog = None


def _build_program():
    import concourse.tile as tile
    from concourse import bacc, mybir
    from concourse.masks import make_identity

    f32 = mybir.dt.float32
    f16 = mybir.dt.float16
    fp8 = mybir.dt.float8e4
    u16 = mybir.dt.uint16
    u32 = mybir.dt.uint32
    ACT = mybir.ActivationFunctionType
    ALU = mybir.AluOpType
    MM = mybir.MatmulPerfMode

    nc = bacc.Bacc("TRN2", target_bir_lowering=False, debug=False)
    s_in = nc.dram_tensor("s16", [P, NT, D], f16, kind="ExternalInput")
    b_in = nc.dram_tensor("bits", [P, MT, N // 16], u16, kind="ExternalInput")
    out = nc.dram_tensor("out", [NH, N], f16, kind="ExternalOutput")

    with tile.TileContext(nc) as tc:
        with tc.tile_pool(name="const", bufs=1) as cpool:
            ident16 = cpool.tile([P, P], f16)
            make_identity(nc, ident16[:])
            identm = cpool.tile([P, P], fp8)
            make_identity(nc, identm[:])
            # fold identity scaled by -120: mask bytes are fp8 2.0 -> adds -240
            nc.vector.tensor_scalar_mul(out=identm[:], in0=identm[:], scalar1=-120.0)
            # warm the sigmoid ACT table once; nothing else touches the tables
            warm = cpool.tile([P, 1], f16)
            nc.scalar.activation(out=warm[:], in_=ident16[:, 0:1], func=ACT.Sigmoid)
            # S_hat.T in fp8, D split into 2 chunks paired for DoubleRow
            stp = cpool.tile([P, 2, N], fp8)
            # all mask bits resident: row m*128+p -> bitsb[p, m, :]
            bitsb = cpool.tile([P, MT, N // 16], u16)

            with (
                tc.tile_pool(name="prep", bufs=1) as prep,
                tc.tile_pool(name="prep_sc", bufs=4) as prep_sc,
                tc.tile_pool(name="outp", bufs=6) as outp,
                tc.tile_pool(name="mmps", bufs=2, space="PSUM") as mmps,
            ):
                s_sb = prep.tile([P, NT, D], f16)
                sh16 = prep.tile([P, NT, D], f16)
                stats = prep.tile([P, NT // 2, 6], f32)
                nsqall = prep.tile([P, NT], f32)
                inv = prep.tile([P, NT], f32)
                maddall = prep.tile([P, MT, N // 2], u16)
                # mask bits ride the scalar HWDGE queue, s chunks the sync one
                nc.scalar.dma_start(out=bitsb[:], in_=b_in[:, :, :])
                nc.sync.dma_start(out=s_sb[:, 0:16, :], in_=s_in[:, 0:16, :])
                nc.sync.dma_start(out=s_sb[:, 16:32, :], in_=s_in[:, 16:32, :])

                def emit_bn_stats(grp):
                    """norm-squares for one group on the vector engine"""
                    t0 = grp * GRP
                    for i in range(GRP):
                        nc.vector.bn_stats(
                            out=stats[:, t0 + i, :], in_=s_sb[:, t0 + i, :]
                        )
                    sl = slice(t0, t0 + GRP)
                    me2 = prep_sc.tile([P, GRP], f32, tag="me2")
                    nc.vector.tensor_tensor(
                        out=me2[:], in0=stats[:, sl, 1], in1=stats[:, sl, 1],
                        op=ALU.mult,
                    )
                    mo2 = prep_sc.tile([P, GRP], f32, tag="mo2")
                    nc.vector.tensor_tensor(
                        out=mo2[:], in0=stats[:, sl, 4], in1=stats[:, sl, 4],
                        op=ALU.mult,
                    )
                    nc.vector.tensor_tensor(
                        out=me2[:], in0=me2[:], in1=mo2[:], op=ALU.add
                    )
                    nc.vector.tensor_tensor(
                        out=nsqall[:, sl], in0=stats[:, sl, 2], in1=stats[:, sl, 5],
                        op=ALU.add,
                    )
                    nc.vector.scalar_tensor_tensor(
                        out=nsqall[:, sl], in0=me2[:], scalar=float(D // 2),
                        in1=nsqall[:, sl], op0=ALU.mult, op1=ALU.add,
                    )

                def emit_sq_stats(grp):
                    """norm-squares for one group on the scalar engine (Square
                    is resident in the sigmoid table set: no table switch)"""
                    t0 = grp * GRP
                    for i in range(GRP):
                        sq = prep_sc.tile([P, D], f16, tag="sq")
                        nc.scalar.activation(
                            out=sq[:], in_=s_sb[:, t0 + i, :], func=ACT.Square,
                            accum_out=nsqall[:, t0 + i : t0 + i + 1],
                        )

                def finish_group(grp):
                    """Quake rsqrt + scale-cast for one group (vector engine)"""
                    t0 = grp * GRP
                    sl = slice(t0, t0 + GRP)
                    # y0 = bitcast((0x5f37 - (bits(nsq) >> 17)) << 16); both the
                    # <<16 and the subtract fold into one exact f32 affine op
                    ybits = prep_sc.tile([P, GRP], u32, tag="ybits")
                    nc.vector.tensor_scalar(
                        out=ybits[:], in0=nsqall[:, sl].bitcast(u32), scalar1=17,
                        scalar2=None, op0=ALU.logical_shift_right,
                    )
                    nc.vector.tensor_scalar(
                        out=ybits[:], in0=ybits[:], scalar1=-65536.0,
                        scalar2=float(0x5F370000), op0=ALU.mult, op1=ALU.add,
                    )
                    y = ybits[:].bitcast(f32)
                    t = prep_sc.tile([P, GRP], f32, tag="t")
                    for _ in range(2):  # Newton: y *= 1.5 - 0.5*nsq*y^2
                        nc.vector.tensor_tensor(out=t[:], in0=y, in1=y, op=ALU.mult)
                        nc.vector.tensor_tensor(
                            out=t[:], in0=t[:], in1=nsqall[:, sl], op=ALU.mult
                        )
                        nc.vector.tensor_scalar(
                            out=t[:], in0=t[:], scalar1=-0.5, scalar2=1.5,
                            op0=ALU.mult, op1=ALU.add,
                        )
                        nc.vector.tensor_tensor(
                            out=inv[:, sl], in0=y, in1=t[:], op=ALU.mult
                        )
                        y = inv[:, sl]
                    for i in range(GRP):
                        nc.vector.tensor_scalar_mul(
                            out=sh16[:, t0 + i, :],
                            in0=s_sb[:, t0 + i, :],
                            scalar1=inv[:, t0 + i : t0 + i + 1],
                        )

                def emit_transposes(grp):
                    t0 = grp * GRP
                    for i in range(2):
                        tps = mmps.tile([P, GRP, P], f16, tag="ps")
                        for tt in range(GRP):
                            nc.tensor.transpose(
                                tps[:, tt, :],
                                sh16[:, t0 + tt, i * P : (i + 1) * P],
                                ident16[:],
                            )
                        nc.vector.tensor_copy(
                            out=stp[:, i, t0 * P : (t0 + GRP) * P], in_=tps[:]
                        )

                def emit_expand(m0, m1, eng=None):
                    """expand bit-planes into fp8 {0,2.0} bytes for tiles [m0,m1)"""
                    ve = eng if eng is not None else nc.vector
                    for k in range(8):
                        dst = maddall[:, m0:m1, k * (N // 16) : (k + 1) * (N // 16)]
                        src = bitsb[:, m0:m1, :]
                        if k == 6:
                            ve.tensor_scalar(
                                out=dst, in0=src, scalar1=0x4040,
                                scalar2=None, op0=ALU.bitwise_and,
                            )
                        elif k < 6:
                            ve.tensor_scalar(
                                out=dst, in0=src, scalar1=6 - k,
                                scalar2=0x4040, op0=ALU.logical_shift_left,
                                op1=ALU.bitwise_and,
                            )
                        else:
                            ve.tensor_scalar(
                                out=dst, in0=src, scalar1=k - 6,
                                scalar2=0x4040, op0=ALU.logical_shift_right,
                                op1=ALU.bitwise_and,
                            )

                def emit_run(m, qs, split=1):
                    """grams+folds+sigmoid+store for 512-col chunks qs (global
                    chunk ids, contiguous); psum/out offsets relative to qs[0]"""
                    lhsT = stp[:, :, m * P : (m + 1) * P]
                    ps = mmps.tile([P, 2048], f32, tag="ps")
                    for qi, q in enumerate(qs):
                        c0 = q * 512
                        nc.tensor.matmul(
                            ps[:, qi * 512 : (qi + 1) * 512],
                            lhsT=lhsT,
                            rhs=stp[:, :, c0 : c0 + 512],
                            start=True,
                            stop=False,
                            perf_mode=MM.DoubleRow,
                        )
                    for qi, q in enumerate(qs):
                        c0 = q * 512
                        nc.tensor.matmul(
                            ps[:, qi * 512 : (qi + 1) * 512],
                            lhsT=identm[:],
                            rhs=maddall[:, m, c0 // 2 : c0 // 2 + 256].bitcast(fp8),
                            start=False,
                            stop=True,
                        )
                    ot = outp.tile([P, 2048], f16, tag="ot")
                    wtot = len(qs) * 512
                    w = wtot // split
                    for sp in range(split):
                        nc.scalar.activation(
                            out=ot[:, sp * w : (sp + 1) * w],
                            in_=ps[:, sp * w : (sp + 1) * w],
                            func=ACT.Sigmoid,
                        )
                        nc.sync.dma_start(
                            out=out[
                                m * P : (m + 1) * P,
                                qs[0] * 512 + sp * w : qs[0] * 512 + (sp + 1) * w,
                            ],
                            in_=ot[:, sp * w : (sp + 1) * w],
                        )

                def emit_half(m, g, split=1):
                    emit_run(m, [g * 4, g * 4 + 1, g * 4 + 2, g * 4 + 3], split)

                # ---- emission schedule ----
                emit_bn_stats(0)
                emit_sq_stats(1)
                emit_sq_stats(2)
                emit_sq_stats(3)
                finish_group(0)
                emit_expand(0, 3)
                emit_transposes(0)
                finish_group(1)
                emit_transposes(1)
                emit_run(0, [0, 1])
                emit_run(1, [0, 1])
                emit_run(2, [0, 1])
                emit_run(0, [2, 3])
                emit_run(1, [2, 3])
                emit_run(2, [2, 3])
                emit_expand(3, 7)
                finish_group(2)
                finish_group(3)
                emit_half(3, 0)
                emit_half(4, 0)
                emit_transposes(2)
                emit_half(5, 0)
                emit_half(6, 0)
                emit_transposes(3)
                emit_half(0, 1)
                emit_half(1, 1)
                emit_half(2, 1)
                emit_expand(7, 11)
                emit_half(3, 1)
                emit_half(4, 1)
                emit_half(5, 1)
                emit_half(6, 1)
                emit_expand(11, MT)
                for m in range(7, MT):
                    emit_half(m, 0)
                    emit_half(m, 1, split=4 if m == MT - 1 else 1)

    nc.compile()
    return nc


def _host_prep(prop_state, mask):
    prop = np.asarray(prop_state)
    mk = np.asarray(mask)
    i = mk[..., 0].astype(np.int64)
    j = mk[..., 1].astype(np.int64)
    # dense edge indicator per batch, as flat bool
    edge = np.zeros((B, N * N), dtype=bool)
    for b in range(B):
        edge[b][i[b] * N + j[b]] = True
        edge[b][j[b] * N + i[b]] = True
    edge = edge.reshape(B, N, N)
    prop16 = prop.astype(np.float16)

    in_maps = []
    for c in range(8):
        b, h = divmod(c, 2)
        r = h * NH
        s_roll = prop16[b] if r == 0 else np.roll(prop16[b], -r, axis=0)
        ne = ~edge[b][r : r + NH]
        if r:
            ne = np.roll(ne, -r, axis=1)
        # byte c bit k = nonedge(row, k*512 + c); u16 = little-endian byte pair
        bits = np.packbits(
            ne.reshape(NH, 8, N // 8), axis=1, bitorder="little"
        ).reshape(NH, N // 8)
        # partition-major device layouts: contiguous per-partition DMA runs
        s_pm = s_roll.reshape(NT, P, D).transpose(1, 0, 2)
        bits_pm = bits.view("<u2").reshape(MT, P, N // 16).transpose(1, 0, 2)
        in_maps.append(
            {
                "s16": np.ascontiguousarray(s_pm),
                "bits": np.ascontiguousarray(bits_pm),
            }
        )
    return in_maps


def _assemble(results):
    outf = np.empty((B, N, N), dtype=np.float32)
    for c in range(8):
        b, h = divmod(c, 2)
        r = h * NH
        o = results[c]["out"].astype(np.float32)
        outf[b, r : r + NH, :] = o if r == 0 else np.roll(o, r, axis=1)
    return outf


def kernel(prop_state, mask):
    from concourse.bass_utils import run_bass_kernel_spmd

    global _prog
    if _prog is None:
        _prog = _build_program()
    in_maps = _host_prep(prop_state, mask)
    res = run_bass_kernel_spmd(_prog, in_maps, core_ids=list(range(8)))
    return _assemble(res.results)
